# revision 52
# baseline (speedup 1.0000x reference)
"""Trainium2 Bass kernel for a 2-layer GCN + global mean pool + MLP head.

Model (reference semantics):
    h1  = relu(GCNConv(x, W1, b1))          # symmetric-normalized A+I aggregation
    h2  = GCNConv(h1, W2, b2)
    g   = global_mean_pool(h2, batch)        # 512 graphs
    out = (relu(g @ W3 + b3) @ W4 + b4)      # [512]

Distribution: nodes (and their in-edges) sharded contiguously across the 8
NeuronCores; the per-layer scaled feature table hs = (x @ W) * dinv[src] is
all-gathered (bf16, 256B-padded rows) so every core can gather messages for
its local destination nodes; edge aggregation is a one-hot (selection-matrix)
matmul accumulated in PSUM per 128-destination tile; pooled sums are
all-reduced and the tiny MLP head runs replicated.

Key design points (vs the naive version):
  - degrees/dinv, per-graph pool counts, and b2-folding (b3' = b2@W3 + b3)
    are precomputed on the host — no device degree pass at all;
  - self-loops are appended to the edge list on the host, so aggregation is
    one uniform gather+matmul stream (no separate self term);
  - one-hot selection matrices are built in batches of up to MAXCH chunks
    with a single broadcast-AP tensor_tensor per gather call;
  - everything matmul-heavy is bf16 (FWL fast weight load);
  - relu+bias of layer 1 runs on the Scalar engine straight out of PSUM.
"""

import numpy as np

P = 128
DIM = 64


class CFG:
    def __init__(self, n=100000, e=1600000, g=512, cores=8, maxch_call=32):
        self.N = n
        self.E = e
        self.G = g
        self.R = cores
        self.PC = -(-n // cores)            # nodes per core (pre-pad)
        self.PC = -(-self.PC // P) * P      # pad to 128
        self.T = self.PC // P               # dst tiles per core
        self.NP = self.PC * cores           # padded global nodes
        self.BANK = 32768
        self.NB = -(-self.NP // self.BANK)  # index banks
        self.MAXCH = maxch_call             # max chunks per gather call
        self.nqueues = 4
        self.Q = 7                          # AllGather split factor
        assert (self.T % self.Q) == 0
        self.replicate_l1 = False           # True: no AG1, all cores build
        self.repeat = 1                     # the full h1 table themselves


FULL = CFG()


# ---------------------------------------------------- host preprocessing ---

def _prep(edge_index, batch, cfg):
    """Partition/pad edges (incl. self-loops); build per-core device arrays
    + shared schedule; host-precompute dinv and pool counts."""
    c = cfg
    src = np.asarray(edge_index[0], dtype=np.int64)
    dst = np.asarray(edge_index[1], dtype=np.int64)
    batch = np.asarray(batch, dtype=np.int64)

    deg = np.bincount(dst, minlength=c.N).astype(np.float64) + 1.0
    dinv_full = (1.0 / np.sqrt(deg)).astype(np.float32)      # [N]

    loop = np.arange(c.N, dtype=np.int64)
    src = np.concatenate([src, loop])
    dst = np.concatenate([dst, loop])

    core = dst // c.PC
    tloc = (dst % c.PC) // P
    slot = dst % P

    # global table rows are permuted to (q-th local segment, core, local)
    # so the layer-2 AllGather splits into Q contiguous-output collectives
    H = c.PC // c.Q
    sr, si = src // c.PC, src % c.PC
    trow = (si // H) * (c.NP // c.Q) + sr * H + (si % H)
    bank = trow // c.BANK
    ib = trow % c.BANK

    rows = np.arange(c.NP, dtype=np.int64)
    rr = (rows % (c.NP // c.Q)) // H
    node_of_row = rr * c.PC + (rows // (c.NP // c.Q)) * H \
        + (rows % (c.NP // c.Q)) % H                     # inverse permutation

    key = (core * c.T + tloc) * c.NB + bank
    order = np.lexsort((ib, key))
    key_s = key[order]
    ib_s = ib[order].astype(np.int16)
    slot_s = slot[order].astype(np.float32)

    nkey = c.R * c.T * c.NB
    cnts = np.bincount(key_s, minlength=nkey)
    counts = cnts.reshape(c.R, c.T, c.NB)
    starts_flat = np.concatenate([[0], np.cumsum(cnts)])

    C_tb = -(-counts.max(axis=0) // P)          # [T, NB]
    for t in range(c.T):
        if C_tb[t].sum() == 0:
            C_tb[t, 0] = 1

    groups = []
    t0 = 0
    while t0 < c.T:
        t1 = t0
        while t1 < c.T:
            nch = C_tb[t0:t1 + 1].sum(axis=0).max()
            if nch > c.MAXCH and t1 > t0:
                break
            t1 += 1
        groups.append((t0, t1))
        t0 = t1

    chunk_of = np.zeros((c.T, c.NB), dtype=np.int64)
    calls = []       # (bank, t0, t1, chunk0, nch, idx_col0)
    CH = 0
    icol = 0
    for (t0, t1) in groups:
        for b in range(c.NB):
            ch0 = CH
            for t in range(t0, t1):
                chunk_of[t, b] = CH
                CH += int(C_tb[t, b])
            nch = CH - ch0
            if nch:
                calls.append((b, t0, t1, ch0, int(nch), icol))
                icol += nch * P // 16
    sched = dict(C_tb=C_tb, chunk_of=chunk_of, groups=groups, calls=calls,
                 CH=int(CH), ICOLS=int(icol))

    # pool counts (host): 1/count per graph, laid out [P, G//P]
    cnt_g = np.bincount(batch, minlength=c.G).astype(np.float32)
    inv_cnt = 1.0 / np.maximum(cnt_g, 1.0)
    NQ = -(-c.G // P)
    invc = np.zeros(NQ * P, dtype=np.float32)
    invc[:c.G] = inv_cnt
    invc = invc.reshape(NQ, P).T.copy()                      # [128, NQ]

    TF = c.NP // P
    dfull = np.ones(c.NP, dtype=np.float32)
    dfull[:c.N] = dinv_full
    dfull = dfull[node_of_row]                               # permuted rows
    dinvF = dfull.reshape(TF, P).T.copy()                    # [128, TF]

    per_core = []
    for r in range(c.R):
        idxw = np.zeros((P, icol), dtype=np.int16)
        dstl = np.full((P, CH), -1.0, dtype=np.float32)
        for (b, t0, t1, ch0, nch, col0) in calls:
            li = np.zeros(nch * P, dtype=np.int16)
            for t in range(t0, t1):
                k = (r * c.T + t) * c.NB + b
                s0, s1 = starts_flat[k], starts_flat[k + 1]
                n = int(s1 - s0)
                if n == 0:
                    continue
                o = int(chunk_of[t, b] - ch0) * P
                li[o:o + n] = ib_s[s0:s1]
                cpos = int(chunk_of[t, b])
                ii = np.arange(n)
                dstl[ii % P, cpos + ii // P] = slot_s[s0:s1]
            w = li.reshape(-1, 16).T                      # [16, ncol]
            idxw[:, col0:col0 + nch * P // 16] = np.tile(w, (8, 1))
        n0 = r * c.PC
        nreal = max(0, min(c.N - n0, c.PC))
        bat = np.full(c.PC, -1.0, dtype=np.float32)
        dpc = np.ones(c.PC, dtype=np.float32)
        if nreal > 0:
            bat[:nreal] = batch[n0:n0 + nreal].astype(np.float32)
            dpc[:nreal] = dinv_full[n0:n0 + nreal]
            g0 = int(batch[n0])
            ghi = int(batch[min(n0 + nreal, c.N) - 1])
            assert ghi - g0 < P, (r, g0, ghi)
        else:
            g0 = c.G - 1
        batchF = bat.reshape(c.T, P).T.copy()             # [128, T]
        dinvP = dpc.reshape(c.T, P).T.copy()              # [128, T]
        per_core.append(dict(idxw=idxw, dstl=dstl, batchF=batchF,
                             dinvP=dinvP, dinvF=dinvF, invc=invc, g0=g0,
                             node_of_row=node_of_row, dfull_n=dinv_full))
    return sched, per_core


# ------------------------------------------------------- program builder ---

def build_program(cfg, sched):
    import concourse.bass as bass
    import concourse.bacc as bacc
    import concourse.mybir as mybir
    import concourse.tile as tile
    from concourse.tile import add_dep_helper

    c = cfg
    dt = mybir.dt
    f32 = dt.float32
    bf = dt.bfloat16
    ROWE = P                                 # 256B bf16 table rows
    C_tb, chunk_of, calls = sched["C_tb"], sched["chunk_of"], sched["calls"]
    CH, ICOLS = sched["CH"], sched["ICOLS"]
    T, NB = c.T, c.NB
    NQ = -(-c.G // P)
    GTOT = -(-(c.G + P) // P) * P

    def tile_chunks(t):
        return [(b, k) for b in range(NB) for k in range(int(C_tb[t, b]))]

    def call_chunk0(t, b):
        for (bb, tt0, tt1, c0, nn, _c) in calls:
            if bb == b and tt0 <= t < tt1:
                return c0
        raise AssertionError((t, b))

    nc = bacc.Bacc("TRN2", target_bir_lowering=False, debug=False,
                   num_devices=c.R, num_swdge_queues=c.nqueues)

    TF = c.NP // P                           # full-table tiles (all cores)

    # ---- I/O ----
    if c.replicate_l1:
        xT_p = nc.declare_dram_parameter("xTb", [DIM, c.NP], bf,
                                         isOutput=False)
    else:
        xT_p = nc.declare_dram_parameter("xTl", [DIM, c.PC], bf,
                                         isOutput=False)
    W1_p = nc.declare_dram_parameter("W1b", [DIM, DIM], bf, isOutput=False)
    W2_p = nc.declare_dram_parameter("W2b", [DIM, DIM], bf, isOutput=False)
    W3_p = nc.declare_dram_parameter("W3f", [DIM, DIM], f32, isOutput=False)
    W4_p = nc.declare_dram_parameter("W4f", [DIM, 1], f32, isOutput=False)
    b1_p = nc.declare_dram_parameter("b1c", [DIM, 1], f32, isOutput=False)
    b3_p = nc.declare_dram_parameter("b3p", [1, DIM], f32, isOutput=False)
    b4_p = nc.declare_dram_parameter("b4r", [1, 1], f32, isOutput=False)
    iotaM_p = nc.declare_dram_parameter("iotaM", [P, P], f32, isOutput=False)
    id128_p = nc.declare_dram_parameter("id128", [P, P], f32, isOutput=False)
    ones1_p = nc.declare_dram_parameter("ones1", [1, P], f32, isOutput=False)
    idxw_p = nc.declare_dram_parameter("idxw", [P, ICOLS], dt.int16,
                                       isOutput=False)
    dstl_p = nc.declare_dram_parameter("dstl", [P, CH], bf, isOutput=False)
    batchF_p = nc.declare_dram_parameter("batchF", [P, T], f32, isOutput=False)
    dinvP_p = nc.declare_dram_parameter("dinvP", [P, T], f32, isOutput=False)
    dinvF_p = nc.declare_dram_parameter("dinvF", [P, TF], f32, isOutput=False)
    invc_p = nc.declare_dram_parameter("invc", [P, NQ], f32, isOutput=False)
    g0f_p = nc.declare_dram_parameter("g0f", [1, 1], f32, isOutput=False)
    g0i_p = nc.declare_dram_parameter("g0i", [1, 1], dt.uint32, isOutput=False)
    pred_p = nc.declare_dram_parameter("pred", [c.G, 1], f32, isOutput=True)

    # ---- internal DRAM (per repeat for clean timing isolation) ----
    hsl_a, hs1l_a, tabs_a, pool_loc_a, pool_red_a = [], [], [], [], []
    for r_ in range(c.repeat):
        hsl_a.append(nc.dram_tensor(f"hs2_local_{r_}", [c.PC, ROWE], bf))
        hs1l_a.append(None if c.replicate_l1 else
                      nc.dram_tensor(f"hs1_local_{r_}", [c.PC, ROWE], bf))
        tab1 = (nc.dram_tensor(f"hs1_table_{r_}", [c.NP, ROWE], bf)
                if c.replicate_l1 else
                nc.dram_tensor(f"hs1_table_{r_}", [c.NP, ROWE], bf,
                               addr_space="Shared"))
        tabs_a.append([tab1,
                       nc.dram_tensor(f"hs2_table_{r_}", [c.NP, ROWE], bf,
                                      addr_space="Shared")])
        pool_loc_a.append(nc.dram_tensor(f"pool_local_{r_}", [GTOT, DIM], f32))
        pool_red_a.append(nc.dram_tensor(f"pool_red_{r_}", [GTOT, DIM],
                                         f32, addr_space="Shared"))

    rg = [list(range(c.R))]

    with tile.TileContext(nc) as tc:
        with (
            tc.tile_pool(name="const", bufs=1) as cpool,
            tc.tile_pool(name="msg", bufs=7) as mpool,
            tc.tile_pool(name="oh", bufs=9) as ohpool,
            tc.tile_pool(name="work", bufs=4) as wpool,
            tc.tile_pool(name="seg", bufs=2) as spool,
            tc.tile_pool(name="ps_agg", bufs=3, space="PSUM") as pagg,
            tc.tile_pool(name="ps_h", bufs=2, space="PSUM") as ph,
            tc.tile_pool(name="ps_tr", bufs=2, space="PSUM") as ptr,
            tc.tile_pool(name="ps_pool", bufs=1, space="PSUM") as ppool,
        ):

            def load(pool, ap, shape, dtype=f32, name=None):
                t = pool.tile(shape, dtype, tag=name)
                nc.sync.dma_start(out=t[:], in_=ap)
                return t

            iotaM = load(cpool, iotaM_p[:, :], [P, P], name="iotaM")
            id128 = load(cpool, id128_p[:, :], [P, P], name="id128")
            ones1 = load(cpool, ones1_p[:, :], [1, P], name="ones1")
            W1b = load(cpool, W1_p[:, :], [DIM, DIM], bf, name="W1b")
            W2b = load(cpool, W2_p[:, :], [DIM, DIM], bf, name="W2b")
            W3f = load(cpool, W3_p[:, :], [DIM, DIM], f32, name="W3f")
            W4f = load(cpool, W4_p[:, :], [DIM, 1], f32, name="W4f")
            b1c = load(cpool, b1_p[:, :], [DIM, 1], name="b1c")
            b3p = load(cpool, b3_p[:, :], [1, DIM], name="b3p")
            b4r = load(cpool, b4_p[:, :], [1, 1], name="b4r")
            idxw = load(cpool, idxw_p[:, :], [P, ICOLS], dt.int16, name="idxw")
            dstlb = load(cpool, dstl_p[:, :], [P, CH], bf, name="dstl")
            batchF = load(cpool, batchF_p[:, :], [P, T], name="batchF")
            dinvP = load(cpool, dinvP_p[:, :], [P, T], name="dinvP")
            dinvF = load(cpool, dinvF_p[:, :], [P, TF], name="dinvF")
            invc = load(cpool, invc_p[:, :], [P, NQ], name="invc")
            g0f = load(cpool, g0f_p[:, :], [1, 1], name="g0f")

            iotaE = cpool.tile([P, P], bf, tag="iotaE")
            nc.vector.tensor_copy(out=iotaE[:], in_=iotaM[:])

            # broadcast mats via PE outer product: ones1.T @ row
            def bcast_row(row_ap, w, name):
                pb = ptr.tile([P, w], f32, tag="tr")
                nc.tensor.matmul(out=pb[:], lhsT=ones1[:], rhs=row_ap,
                                 start=True, stop=True)
                sb = cpool.tile([P, w], f32, tag=name)
                nc.vector.tensor_copy(out=sb[:], in_=pb[:])
                return sb

            bB3 = bcast_row(b3p[:], DIM, "bB3")
            bB4 = bcast_row(b4r[:], 1, "bB4")
            g0B = bcast_row(g0f[:], 1, "g0B")

            seg_pads = set()        # (tag, buf-idx) pads already zeroed

            def seg_slot(pool, tag, nseg, idx, nbufs):
                """Segment-accumulation tile [P, nseg, ROWE]; zero the 256B-row
                pad region only the first time each ring buffer is used."""
                sseg = pool.tile([P, nseg, ROWE], bf, tag=tag)
                key = (tag, idx % nbufs)
                if key not in seg_pads:
                    seg_pads.add(key)
                    nc.vector.memset(sseg[:, :, DIM:ROWE], 0.0)
                return sseg

            def seg_store(dram, s0, nseg, sseg):
                """One DMA for nseg tiles: rows (s0+j)*128+p <- sseg[p, j, :]."""
                out_ap = dram[s0 * P:(s0 + nseg) * P, :].rearrange(
                    "(j p) e -> p j e", p=P)
                return nc.sync.dma_start(out=out_ap, in_=sseg[:])

            for _rep in range(c.repeat):
                hs2l_r = hsl_a[_rep]
                tabs_r = tabs_a[_rep]
                pool_loc_r = pool_loc_a[_rep]
                pool_red_r = pool_red_a[_rep]

                # ---------- layer-1 features ----------
                # replicate_l1: every core computes xs@W1 for ALL nodes
                # (xs = dinv*x host-folded) and writes the full bf16 table
                # locally — no AllGather for layer 1.
                # else: compute only local tiles, stage to hs1_local, and
                # AllGather in Q split parts (same permuted row layout).
                bank_w = [[] for _ in range(NB)]
                ag1_parts = []
                NT1 = TF if c.replicate_l1 else T
                SEG = 32 if c.replicate_l1 else 7
                FB = 4                       # feature tiles per PSUM batch
                QT1 = T // c.Q
                with tc.tile_pool(name="xTp", bufs=2) as xpool:
                    for s0 in range(0, NT1, SEG):
                        s1 = min(s0 + SEG, NT1)
                        xt = xpool.tile([DIM, (s1 - s0) * P], bf, tag="xseg")
                        nc.sync.dma_start(out=xt[:],
                                          in_=xT_p[:, s0 * P:s1 * P])
                        sseg = seg_slot(spool, "sseg", s1 - s0, s0 // SEG, 2)
                        for tq in range(s0, s1, FB):
                            nb = min(FB, s1 - tq)
                            hp = ph.tile([P, FB, DIM], f32, tag="h")
                            for t in range(tq, tq + nb):
                                nc.tensor.matmul(
                                    out=hp[:, t - tq, :],
                                    lhsT=xt[:, (t - s0) * P:(t - s0 + 1) * P],
                                    rhs=W1b[:], start=True, stop=True)
                            nc.scalar.activation(
                                out=sseg[:, tq - s0:tq - s0 + nb, 0:DIM],
                                in_=hp[:, 0:nb, :],
                                func=mybir.ActivationFunctionType.Copy)
                        if c.replicate_l1:
                            w = seg_store(tabs_r[0], s0, s1 - s0, sseg)
                            bank_w[s0 * P // c.BANK].append(w)
                            if (s1 * P - 1) // c.BANK != s0 * P // c.BANK:
                                bank_w[(s1 * P - 1) // c.BANK].append(w)
                        else:
                            seg_store(hs1l_a[_rep], s0, s1 - s0, sseg)
                            if s1 % QT1 == 0:
                                qi = s1 // QT1 - 1
                                NPQ = c.NP // c.Q
                                ag1_parts.append(
                                    nc.gpsimd.collective_compute(
                                        "AllGather", mybir.AluOpType.bypass,
                                        replica_groups=rg,
                                        ins=[hs1l_a[_rep][qi * QT1 * P:
                                                          (qi + 1) * QT1 * P,
                                                          :]],
                                        outs=[tabs_r[0][qi * NPQ:
                                                        (qi + 1) * NPQ, :]]))

                # ---------- edge-aggregation layer ----------
                def run_layer(tab, deps_of_bank, emit_tile_out):
                    gat_of = {}
                    for ci, (b, t0, t1, ch0, nchv, col0) in enumerate(calls):
                        m = mpool.tile([P, nchv, ROWE], bf, tag="msg")
                        g = nc.gpsimd.dma_gather(
                            m[:], tab[b * c.BANK:min((b + 1) * c.BANK, c.NP), :],
                            idxw[:, col0:col0 + nchv * P // 16], nchv * P,
                            nchv * P, ROWE,
                            single_packet=(nchv * P <= 1024),
                            queue_num=ci % c.nqueues)
                        for dep in deps_of_bank(b):
                            add_dep_helper(g.ins, dep.ins)
                        oh = ohpool.tile([P, nchv, P], bf, tag="oh")
                        nc.vector.tensor_tensor(
                            out=oh[:],
                            in0=iotaE[:].unsqueeze(1).broadcast_to(
                                [P, nchv, P]),
                            in1=dstlb[:, ch0:ch0 + nchv].unsqueeze(2)
                                .broadcast_to([P, nchv, P]),
                            op=mybir.AluOpType.is_equal)
                        for t in range(t0, t1):
                            gat_of[(t, b)] = (m, oh)
                    for t in range(T):
                        chunks = tile_chunks(t)
                        ap = pagg.tile([P, DIM], f32, tag="agg")
                        for j, (b, k) in enumerate(chunks):
                            m, oh = gat_of[(t, b)]
                            kk = int(chunk_of[t, b]) - call_chunk0(t, b) + k
                            nc.tensor.matmul(
                                out=ap[:], lhsT=oh[:, kk, :],
                                rhs=m[:, kk, 0:DIM],
                                start=(j == 0), stop=(j == len(chunks) - 1))
                        emit_tile_out(t, ap)

                # layer-1 tile epilogue: scale -> transpose -> relu+bias ->
                # h2 -> scaled table row (batched into SEG2-tile stores);
                # after each T/Q block, AllGather that block of the table
                # so communication overlaps the remaining aggregation.
                SEG2 = 7
                QT = T // c.Q
                l1_seg = [None]
                ag_parts = []

                def l1_out(t, ap):
                    j = t % SEG2
                    if j == 0:
                        nseg = min(SEG2, T - t)
                        l1_seg[0] = seg_slot(spool, "sseg2", nseg,
                                             t // SEG2, 2)
                    z = wpool.tile([P, DIM], f32, tag="z")
                    nc.vector.tensor_scalar(
                        out=z[:], in0=ap[:], scalar1=dinvP[:, t:t + 1],
                        scalar2=None, op0=mybir.AluOpType.mult)
                    tp = ptr.tile([DIM, P], f32, tag="tr")
                    nc.tensor.transpose(out=tp[:], in_=z[:], identity=id128[:])
                    o1T = wpool.tile([DIM, P], bf, tag="o1T")
                    nc.scalar.activation(
                        out=o1T[:], in_=tp[:],
                        func=mybir.ActivationFunctionType.Relu,
                        bias=b1c[:, 0:1])
                    h2 = ph.tile([P, DIM], f32, tag="h")
                    nc.tensor.matmul(out=h2[:], lhsT=o1T[:], rhs=W2b[:],
                                     start=True, stop=True)
                    nc.scalar.activation(
                        out=l1_seg[0][:, j, 0:DIM], in_=h2[:],
                        func=mybir.ActivationFunctionType.Copy,
                        scale=dinvP[:, t:t + 1])
                    if j == SEG2 - 1 or t == T - 1:
                        s0 = t - j
                        seg_store(hs2l_r, s0, j + 1, l1_seg[0])
                    if (t + 1) % QT == 0:
                        qi = (t + 1) // QT - 1
                        NPQ = c.NP // c.Q
                        ag_parts.append(nc.gpsimd.collective_compute(
                            "AllGather", mybir.AluOpType.bypass,
                            replica_groups=rg,
                            ins=[hs2l_r[qi * QT * P:(qi + 1) * QT * P, :]],
                            outs=[tabs_r[1][qi * NPQ:(qi + 1) * NPQ, :]]))

                def parts_for_bank(parts):
                    NPQ = c.NP // c.Q

                    def f(b):
                        lo, hi = b * c.BANK, min((b + 1) * c.BANK, c.NP)
                        return [p for qi, p in enumerate(parts)
                                if qi * NPQ < hi and (qi + 1) * NPQ > lo]
                    return f

                if c.replicate_l1:
                    run_layer(tabs_r[0], lambda b: bank_w[b], l1_out)
                else:
                    run_layer(tabs_r[0], parts_for_bank(ag1_parts), l1_out)

                psum_pool = ppool.tile([P, DIM], f32, tag="pool")

                def l2_out(t, ap):
                    pt = wpool.tile([P, DIM], f32, tag="poolt")
                    nc.scalar.activation(
                        out=pt[:], in_=ap[:],
                        func=mybir.ActivationFunctionType.Copy,
                        scale=dinvP[:, t:t + 1])
                    og = wpool.tile([P, P], f32, tag="ohg")
                    nc.vector.tensor_scalar(
                        out=og[:], in0=iotaM[:], scalar1=g0B[:, 0:1],
                        scalar2=batchF[:, t:t + 1], op0=mybir.AluOpType.add,
                        op1=mybir.AluOpType.is_equal)
                    nc.tensor.matmul(out=psum_pool[:], lhsT=og[:], rhs=pt[:],
                                     start=(t == 0), stop=(t == T - 1))

                run_layer(tabs_r[1], parts_for_bank(ag_parts), l2_out)

                # ---------- pool finalize + AllReduce ----------
                poolsb = wpool.tile([P, DIM], f32, tag="poolsb")
                nc.vector.tensor_copy(out=poolsb[:], in_=psum_pool[:])
                zt = wpool.tile([P, DIM], f32, tag="zt")
                nc.vector.memset(zt[:], 0.0)
                zdmas = []
                for q in range(GTOT // P):
                    zdmas.append(nc.sync.dma_start(
                        out=pool_loc_r[q * P:(q + 1) * P, :], in_=zt[:]))
                g0reg = nc.sync.alloc_register(f"g0reg{_rep}")
                nc.sync.reg_load(g0reg, g0i_p[0:1, 0:1])
                g0val = nc.sync.snap(g0reg, donate=True, min_val=0,
                                     max_val=GTOT - P)
                wdma = nc.sync.dma_start(
                    out=pool_loc_r[bass.ds(g0val, P), :], in_=poolsb[:])
                for zd in zdmas:
                    add_dep_helper(wdma.ins, zd.ins)
                ar = nc.gpsimd.collective_compute(
                    "AllReduce", mybir.AluOpType.add, replica_groups=rg,
                    ins=[pool_loc_r[:, :]], outs=[pool_red_r[:, :]])
                add_dep_helper(ar.ins, wdma.ins)

                # ---------- head ----------
                for q in range(NQ):
                    gq = min(P, c.G - q * P)
                    S = wpool.tile([P, DIM], f32, tag="S")
                    d = nc.sync.dma_start(out=S[:],
                                          in_=pool_red_r[q * P:q * P + P, :])
                    add_dep_helper(d.ins, ar.ins)
                    gt = wpool.tile([P, DIM], f32, tag="gt")
                    nc.vector.tensor_scalar(out=gt[:], in0=S[:],
                                            scalar1=invc[:, q:q + 1],
                                            scalar2=None,
                                            op0=mybir.AluOpType.mult)
                    tp = ptr.tile([DIM, P], f32, tag="tr")
                    nc.tensor.transpose(out=tp[:], in_=gt[:], identity=id128[:])
                    gT = wpool.tile([DIM, P], f32, tag="gT")
                    nc.vector.tensor_copy(out=gT[:], in_=tp[:])
                    zp = ph.tile([P, DIM], f32, tag="h")
                    nc.tensor.matmul(out=zp[:], lhsT=gT[:], rhs=W3f[:],
                                     start=True, stop=True)
                    zz = wpool.tile([P, DIM], f32, tag="zz")
                    nc.vector.tensor_tensor(out=zz[:], in0=zp[:], in1=bB3[:],
                                            op=mybir.AluOpType.add)
                    nc.vector.tensor_scalar(out=zz[:], in0=zz[:], scalar1=0.0,
                                            scalar2=None,
                                            op0=mybir.AluOpType.max)
                    tp2 = ptr.tile([DIM, P], f32, tag="tr")
                    nc.tensor.transpose(out=tp2[:], in_=zz[:],
                                        identity=id128[:])
                    zT = wpool.tile([DIM, P], f32, tag="zT")
                    nc.vector.tensor_copy(out=zT[:], in_=tp2[:])
                    pp = ptr.tile([P, 1], f32, tag="tr")
                    nc.tensor.matmul(out=pp[:], lhsT=zT[:], rhs=W4f[:],
                                     start=True, stop=True)
                    pr = wpool.tile([P, 1], f32, tag="pr")
                    nc.vector.tensor_tensor(out=pr[:], in0=pp[:], in1=bB4[:],
                                            op=mybir.AluOpType.add)
                    nc.sync.dma_start(out=pred_p[q * P:q * P + gq, :],
                                      in_=pr[:gq, :])
    nc.compile()
    return nc


# --------------------------------------------------------------- runner ---

def _make_in_maps(x, W1, b1, W2, b2, W3, b3, W4, b4, cfg, per_core):
    import ml_dtypes
    bf16 = ml_dtypes.bfloat16
    c = cfg
    iotaM = np.tile(np.arange(P, dtype=np.float32)[None, :], (P, 1))
    id128 = np.eye(P, dtype=np.float32)
    ones1 = np.ones((1, P), dtype=np.float32)
    W1 = np.asarray(W1, np.float32)
    W2 = np.asarray(W2, np.float32)
    W3 = np.asarray(W3, np.float32)
    W4 = np.asarray(W4, np.float32).reshape(DIM, 1)
    b1 = np.asarray(b1, np.float32).reshape(DIM, 1)
    b2 = np.asarray(b2, np.float32).reshape(1, DIM)
    b3 = np.asarray(b3, np.float32).reshape(1, DIM)
    b4 = np.asarray(b4, np.float32).reshape(1, 1)
    b3p = (b2 @ W3 + b3).astype(np.float32)        # fold b2 through W3
    xf = np.zeros((c.NP, DIM), dtype=np.float32)
    # dinv[src] is folded into x on the host: d*(x@W1) == (d*x)@W1
    xf[:c.N] = np.asarray(x, dtype=np.float32) \
        * per_core[0]["dfull_n"][:, None]
    xfp = xf[per_core[0]["node_of_row"]]           # permuted table rows
    xTb = np.ascontiguousarray(xfp.T).astype(bf16)
    maps = []
    for r in range(c.R):
        pc = per_core[r]
        # local nodes of core r in local order (matches hs1_local rows)
        xTl = np.ascontiguousarray(
            xf[r * c.PC:(r + 1) * c.PC].T).astype(bf16)
        maps.append({
            "xTb": xTb, "xTl": xTl,
            "W1b": W1.astype(bf16), "W2b": W2.astype(bf16),
            "W3f": W3, "W4f": W4,
            "b1c": b1, "b3p": b3p, "b4r": b4,
            "iotaM": iotaM, "id128": id128, "ones1": ones1,
            "idxw": pc["idxw"], "dstl": pc["dstl"].astype(bf16),
            "batchF": pc["batchF"],
            "dinvP": pc["dinvP"], "dinvF": pc["dinvF"], "invc": pc["invc"],
            "g0f": np.array([[float(pc["g0"])]], dtype=np.float32),
            "g0i": np.array([[pc["g0"]]], dtype=np.uint32),
        })
    return maps


def kernel(x, edge_index, batch, W1, b1, W2, b2, W3, b3, W4, b4,
           cfg=None, run=None):
    import sys
    if "/opt/trn_rl_repo" not in sys.path:
        sys.path.insert(0, "/opt/trn_rl_repo")
    cfg = cfg or FULL
    x = np.asarray(x)
    edge_index = np.asarray(edge_index)
    batch = np.asarray(batch)
    sched, per_core = _prep(edge_index, batch, cfg)
    nc = build_program(cfg, sched)
    maps = _make_in_maps(x, W1, b1, W2, b2, W3, b3, W4, b4, cfg, per_core)
    if run is not None:                 # custom runner (e.g. simulator)
        return run(nc, maps)
    from concourse.bass_utils import run_bass_kernel_spmd
    res = run_bass_kernel_spmd(nc, maps, list(range(cfg.R)))
    return np.asarray(res.results[0]["pred"]).reshape(-1).astype(np.float32)
